# revision 23
# baseline (speedup 1.0000x reference)
"""Trainium2 Bass kernel for the Conv2d abstract-bound-matrix problem.

Computes, for a 3x3/stride-1/pad-1 conv layer (C_IN=8, C_OUT=16, 32x32):
  - M:  the (8193, 16385) abstract affine matrix (column (c,i,j) holds the
        flattened-input coefficients of output pixel (c,i,j); last row bias,
        last column homogeneous [0..0,1]).
  - lo/hi: interval bounds of the outputs via back-substitution.

Strategy (column sharding, 2 output channels per core, 8 cores):
  M's (cin, c) 1024x1024 block is block-tridiagonal with three repeated 32x32
  Toeplitz blocks T2/T1/T0 (one per kh).  Each 128-row tile of the slab is a
  sliding window over a precomputed SBUF "staircase" image E, so the whole
  67 MB/core slab is materialized by 16 big DMAs (one per (cin,c) block) whose
  source access pattern re-reads overlapping windows of E.  Each DRAM byte is
  written exactly once; no zero-fill pass, no write-after-write hazards.
  lo/hi are computed on-device with TensorE matmuls over im2col'd input bounds.

  All small inputs arrive in ONE packed DRAM tensor (single DMA -> single
  completion semaphore), and E is zeroed only on the complement of the
  staircase rectangles, so no compute instruction ever needs more than one
  semaphore wait (trn2 codegen limit for compute engines).
"""

import os
import sys

import numpy as np

if "/opt/trn_rl_repo" not in sys.path:
    sys.path.insert(0, "/opt/trn_rl_repo")

# Static layer configuration
C_IN, H, W = 8, 32, 32
C_OUT, KH, KW = 16, 3, 3
OH, OW = 32, 32
N_IN = C_IN * H * W        # 8192
N_OUT = C_OUT * OH * OW    # 16384
N_CORES = 8
CPC = C_OUT // N_CORES     # output channels per core = 2
SLAB_COLS = CPC * OH * OW  # 2048
SLAB_ROWS = N_IN + 1       # 8193
NB = C_IN * CPC            # staircase blocks per core = 16
EB = 2048                  # E columns per block
TTW = 96                   # TT tile width = 3 * 32

# Packed input layout (columns within the [128, PACK_W] tensor)
OFF_TT = 0                 # [128, 1536]
OFF_LP = OFF_TT + NB * TTW  # [72, 1024] @ 1536
OFF_UP = OFF_LP + 1024      # [72, 1024] @ 2560
OFF_WP = OFF_UP + 1024      # [72, 2]    @ 3584
OFF_WN = OFF_WP + CPC       # [72, 2]    @ 3586
OFF_B2T = OFF_WN + CPC      # [1, 2]     @ 3588 (bias, partition 0)
OFF_ONES = 3592             # [1, 512]   @ 3592 (ones row, partition 0)
OFF_BROW = 4104             # [1, 2048]  @ 4104
PACK_W = OFF_BROW + SLAB_COLS  # 6152

LAST_RESULTS = None  # BassKernelResults of the most recent run (for test.py)

_PROGRAM = None


def _build_program():
    import concourse.bass as bass
    import concourse.mybir as mybir
    import bass_rust
    from concourse.tile import TileContext
    from concourse.tile_rust import add_dep_helper

    f32 = mybir.dt.float32
    nc = bass.Bass()

    pack_in = nc.dram_tensor("pack", (128, PACK_W), f32, kind="ExternalInput")
    mslab = nc.dram_tensor("mslab", (SLAB_ROWS, SLAB_COLS), f32, kind="ExternalOutput")
    lo_out = nc.dram_tensor("lo", (CPC, 1024), f32, kind="ExternalOutput")
    hi_out = nc.dram_tensor("hi", (CPC, 1024), f32, kind="ExternalOutput")

    with TileContext(nc) as tc:
        with (
            tc.tile_pool(name="big", bufs=1) as big,
            tc.tile_pool(name="const", bufs=1) as constp,
            tc.tile_pool(name="outs", bufs=1) as outp,
            tc.tile_pool(name="psum", bufs=1, space="PSUM") as psum,
        ):
            E_tiles = [
                big.tile([128, EB], f32, tag=f"E{b}", name=f"E{b}")
                for b in range(NB)
            ]
            PACK = constp.tile([128, PACK_W], f32)

            pack_dma = nc.sync.dma_start(out=PACK[:], in_=pack_in[:])

            TT = PACK[:, OFF_TT : OFF_TT + NB * TTW]
            Lp = PACK[0:72, OFF_LP : OFF_LP + 1024]
            Up = PACK[0:72, OFF_UP : OFF_UP + 1024]
            Wp = PACK[0:72, OFF_WP : OFF_WP + CPC]
            Wn = PACK[0:72, OFF_WN : OFF_WN + CPC]
            B2T = PACK[0:1, OFF_B2T : OFF_B2T + CPC]
            Ones = PACK[0:1, OFF_ONES : OFF_ONES + 512]
            Brow = PACK[0:1, OFF_BROW : OFF_BROW + SLAB_COLS]

            # Bias row of the slab (row 8192).  SP lane1, waits lane0 only.
            brow_dma = nc.sync.dma_start(out=mslab[N_IN : N_IN + 1, :], in_=Brow)

            # lo/hi: [CPC,1024] = Wp.T@Lp + Wn.T@Up + b·1^T; hi swaps Lp/Up.
            # The bias lands via a third K=1 matmul so the PSUM->SBUF copy
            # (DVE) waits on PE only -- never two semaphores at once.
            sb_halves = []
            for half in range(2):
                ns = slice(512 * half, 512 * half + 512)
                ps_lo = psum.tile([CPC, 512], f32, tag=f"ps_lo{half}")
                nc.tensor.matmul(ps_lo[:], Wp, Lp[:, ns], start=True, stop=False)
                nc.tensor.matmul(ps_lo[:], Wn, Up[:, ns], start=False, stop=False)
                nc.tensor.matmul(ps_lo[:], B2T, Ones, start=False, stop=True)
                lo_sb = outp.tile([CPC, 512], f32, tag=f"lo_sb{half}")
                nc.vector.tensor_copy(out=lo_sb[:], in_=ps_lo[:])

                ps_hi = psum.tile([CPC, 512], f32, tag=f"ps_hi{half}")
                nc.tensor.matmul(ps_hi[:], Wp, Up[:, ns], start=True, stop=False)
                nc.tensor.matmul(ps_hi[:], Wn, Lp[:, ns], start=False, stop=False)
                last_mm = nc.tensor.matmul(
                    ps_hi[:], B2T, Ones, start=False, stop=True
                )
                hi_sb = outp.tile([CPC, 512], f32, tag=f"hi_sb{half}")
                nc.vector.tensor_copy(out=hi_sb[:], in_=ps_hi[:])
                sb_halves.append((ns, lo_sb, hi_sb))

            # Staircase build, block by block.  Copies first (PACK already
            # observed on DVE -> 0 waits), then complement memsets (disjoint
            # regions -> no deps -> 0 waits).  Blocks 0 and 1 are built LAST
            # so that each DMA engine's first spray (block 0 on SP, block 1
            # on Act) waits for the maximal DVE tick; all later sprays' data
            # waits are subsumed, leaving only the single lane-reuse wait.
            for b in list(range(2, NB)) + [1, 0]:
                E = E_tiles[b]
                for g in range(4):
                    nc.vector.tensor_copy(
                        out=E[32 * g : 32 * g + 32,
                              864 + 32 * g : 960 + 32 * g],
                        in_=TT[32 * g : 32 * g + 32, TTW * b : TTW * b + TTW],
                    )
                # Zero everything except the four staircase rectangles.
                nc.vector.memset(E[:, 0:864], 0.0)
                nc.vector.memset(E[:, 1056:2048], 0.0)
                nc.vector.memset(E[0:32, 960:1056], 0.0)
                nc.vector.memset(E[32:64, 864:896], 0.0)
                nc.vector.memset(E[32:64, 992:1056], 0.0)
                nc.vector.memset(E[64:96, 864:928], 0.0)
                nc.vector.memset(E[64:96, 1024:1056], 0.0)
                last_c = nc.vector.memset(E[96:128, 864:960], 0.0)

            # Sprays: row-tile t reads the window starting at 896-128t;
            # iterate t' = 7-t so the SBUF side has positive strides.
            sprays = []
            for b in range(NB):
                cin, cl = b // CPC, b % CPC
                E = E_tiles[b]
                src = E[:].copy()
                src.ap = bass_rust.VecI64Pair([(EB, 128), (128, 8), (1, 1024)])
                src.offset = 0
                dst = mslab[:].copy()
                dst.ap = bass_rust.VecI64Pair(
                    [(SLAB_COLS, 128), (-128 * SLAB_COLS, 8), (1, 1024)]
                )
                dst.offset = (1024 * cin + 896) * SLAB_COLS + 1024 * cl
                eng = nc.sync if b % 2 == 0 else nc.scalar
                spray = eng.dma_start(out=dst, in_=src)
                # Gate every spray on the end of ALL construction: they all
                # become ready at one tick, so per engine only the first
                # spray emits the DVE wait and later ones carry at most the
                # single lane-reuse wait (walrus allows 1 wait/instruction).
                add_dep_helper(spray.ins, last_c.ins, reason="spray gate")
                sprays.append(spray)

            # lo/hi stores last (DVE ticks long observed -> lane wait only).
            stores = []
            for ns, lo_sb, hi_sb in sb_halves:
                stores.append(nc.sync.dma_start(out=lo_out[:, ns], in_=lo_sb[:]))
                stores.append(nc.sync.dma_start(out=hi_out[:, ns], in_=hi_sb[:]))

            # Observer nops: the SP drain aggregates every proc's final tick,
            # but drains may carry at most one emitted wait.  Chain nops on
            # SP (and Act) so those engines observe every DMA completion,
            # the last matmul, and the last construction op one wait at a
            # time; the drains then have nothing new to wait on.
            tail = [pack_dma, brow_dma, last_mm, last_c] + sprays + stores
            for j, d in enumerate(tail):
                n1 = nc.sync.nop(nofuse=True, hint=f"obs_sp{j}")
                add_dep_helper(n1.ins, d.ins, reason="observe sp")
                n2 = nc.scalar.nop(nofuse=True, hint=f"obs_act{j}")
                add_dep_helper(n2.ins, d.ins, reason="observe act")

    return nc


def _host_prep(weights, bias, concrete_lower, concrete_upper):
    """Build per-core packed input maps (all small; heavy lifting on-device)."""
    w = np.asarray(weights, np.float32)
    bias = np.asarray(bias, np.float32)
    l = np.asarray(concrete_lower, np.float32)
    u = np.asarray(concrete_upper, np.float32)

    # im2col of padded bounds: Lp[cin*9+kh*3+kw, i*32+j] = xpad[cin, i+kh, j+kw]
    def im2col(x):
        xp = np.pad(x, ((0, 0), (1, 1), (1, 1)))
        out = np.empty((72, 1024), np.float32)
        for cin in range(C_IN):
            for kh in range(KH):
                for kw in range(KW):
                    out[cin * 9 + kh * 3 + kw] = xp[
                        cin, kh : kh + 32, kw : kw + 32
                    ].reshape(-1)
        return out

    lp = im2col(l)
    up = im2col(u)
    wpos = np.maximum(w, 0.0)
    wneg = np.minimum(w, 0.0)

    in_maps = []
    for k in range(N_CORES):
        pack = np.zeros((128, PACK_W), np.float32)
        for b in range(NB):
            cin, cl = b // CPC, b % CPC
            wb = w[CPC * k + cl, cin]  # (3,3)
            tt32 = np.zeros((32, TTW), np.float32)
            i = np.arange(31)
            for s, kh in enumerate((2, 1, 0)):  # TT = [T2 | T1 | T0]
                T = tt32[:, 32 * s : 32 * s + 32]
                T[i, i + 1] = wb[kh, 0]
                T[np.arange(32), np.arange(32)] = wb[kh, 1]
                T[i + 1, i] = wb[kh, 2]
            pack[:, OFF_TT + TTW * b : OFF_TT + TTW * (b + 1)] = np.tile(
                tt32, (4, 1)
            )

        cg = slice(CPC * k, CPC * (k + 1))
        pack[0:72, OFF_LP : OFF_LP + 1024] = lp
        pack[0:72, OFF_UP : OFF_UP + 1024] = up
        pack[0:72, OFF_WP : OFF_WP + CPC] = wpos[cg].reshape(CPC, 72).T
        pack[0:72, OFF_WN : OFF_WN + CPC] = wneg[cg].reshape(CPC, 72).T
        pack[0, OFF_B2T : OFF_B2T + CPC] = bias[cg]
        pack[0, OFF_ONES : OFF_ONES + 512] = 1.0
        pack[0, OFF_BROW : OFF_BROW + SLAB_COLS] = np.repeat(bias[cg], OH * OW)
        in_maps.append({"pack": pack})
    return in_maps


def kernel(concrete_lower, concrete_upper, weights, bias):
    global _PROGRAM, LAST_RESULTS
    from concourse import bass_utils

    if _PROGRAM is None:
        _PROGRAM = _build_program()

    in_maps = _host_prep(weights, bias, concrete_lower, concrete_upper)
    res = bass_utils.run_bass_kernel_spmd(
        _PROGRAM, in_maps, core_ids=list(range(N_CORES))
    )
    LAST_RESULTS = res

    M = np.zeros((N_IN + 1, N_OUT + 1), np.float32)
    lo = np.empty((C_OUT, OH * OW), np.float32)
    hi = np.empty((C_OUT, OH * OW), np.float32)
    for k in range(N_CORES):
        out = res.results[k]
        M[:, SLAB_COLS * k : SLAB_COLS * (k + 1)] = out["mslab"]
        lo[CPC * k : CPC * (k + 1)] = out["lo"]
        hi[CPC * k : CPC * (k + 1)] = out["hi"]
    M[N_IN, N_OUT] = 1.0
    return (
        lo.reshape(C_OUT, OH, OW),
        hi.reshape(C_OUT, OH, OW),
        M,
    )
